# revision 24
# baseline (speedup 1.0000x reference)
"""Block 8x8 2D-IDCT kernel for Trainium2 (Bass/Tile), 8-core data-parallel.

Full input x_dct (4,64,64,64,8,8) f32 is sharded along flattened (N,C) into
8 shards of 32 images; each core independently computes the 2D IDCT of its
32 images and writes (32,512,512); results are concatenated on host.

Per-core pipeline, one tile = 2 images = 2 MiB = [128p x 4096] f32; each
partition p = (img, bh) holds one full block-row (32 block-pairs):
  DMA load (16KB/partition contiguous)
  -> PE transpose (float32r: 1.5 cyc/row, bit-exact pass-through) of 32x
     [128,128] sub-tiles (sub-tile s = block-pair column s; partitions
     become the 128 coeffs of the pair)
  -> DVE copy PSUM->SBUF casting to bf16
  -> bf16 matmul per sub-tile against G2 = blockdiag(G^T,G^T) in bf16,
     G = kron(M,M); fp32 PSUM accumulate:
       out[pair, 128 pixels of 2 blocks] in PSUM
  -> ACT copy PSUM->SBUF (fp32) permuted so free dim = (i, s, g, j) = (i, w)
  -> DMA store: 8 stores/tile, 256KB each, full 2KB DRAM rows
"""

import math
from contextlib import ExitStack

import numpy as np

import concourse.bass as bass
import concourse.mybir as mybir
import concourse.tile as tile
from concourse import bacc, masks
from concourse.bass_utils import run_bass_kernel_spmd

F32 = mybir.dt.float32
F32R = mybir.dt.float32r
BF16 = mybir.dt.bfloat16

N_CORES = 8
IMGS = 32           # images per core
TILES = IMGS // 2   # 2 images per tile
P = 128
SUBT = 32           # [128,128] sub-tiles per tile
GRPS = 8            # groups of 4 sub-tiles (one PSUM bank each)
BLOCK = 8


def _make_idct_matrix(nb: int) -> np.ndarray:
    m = np.zeros((nb, nb), dtype=np.float64)
    for n in range(nb):
        for k in range(nb):
            alpha = math.sqrt(1.0 / nb) if k == 0 else math.sqrt(2.0 / nb)
            m[n, k] = alpha * math.cos(math.pi * (2 * n + 1) * k / (2 * nb))
    return m.astype(np.float32)


def _build_nc(tiles: int = TILES) -> bass.Bass:
    nc = bacc.Bacc("TRN2", target_bir_lowering=False, debug=False)

    x = nc.dram_tensor("x", [tiles, P, 4096], F32, kind="ExternalInput")
    g2 = nc.dram_tensor("g2", [P, P], BF16, kind="ExternalInput")
    out = nc.dram_tensor("out", [2 * tiles, 512, 512], F32, kind="ExternalOutput")
    # out view: (t, im, u, i, w)
    outv = out[:].rearrange(
        "(t im) (u i) w -> t im u i w", t=tiles, im=2, u=64, i=8
    )

    with tile.TileContext(nc) as tc, ExitStack() as ctx:
        consts = ctx.enter_context(tc.tile_pool(name="consts", bufs=1))
        lpool = ctx.enter_context(tc.tile_pool(name="load", bufs=6))
        s1pool = ctx.enter_context(tc.tile_pool(name="s1", bufs=6))
        s3pool = ctx.enter_context(tc.tile_pool(name="s3", bufs=6))
        pt = ctx.enter_context(
            tc.tile_pool(name="pt", bufs=3, space=bass.MemorySpace.PSUM)
        )
        po = ctx.enter_context(
            tc.tile_pool(name="po", bufs=3, space=bass.MemorySpace.PSUM)
        )
        pw = ctx.enter_context(
            tc.tile_pool(name="pw", bufs=1, space=bass.MemorySpace.PSUM)
        )

        ident = consts.tile([P, P], F32)
        masks.make_identity(nc, ident[:])
        g2t = consts.tile([P, P], BF16)
        nc.sync.dma_start(g2t[:], g2[:])

        # PE warmup: ~5us of back-to-back matmuls so the systolic array
        # reaches full pstate before tile 0's transposes arrive (a cold
        # PE delays the first stores and cascades into DMA stalls).
        wu = pw.tile([P, P], F32)
        for _ in range(36):
            nc.tensor.matmul(wu[:], g2t[:], g2t[:], start=True, stop=True)

        for t in range(tiles):
            L = lpool.tile([P, 4096], F32)
            # loads alternate between the SP HWDGE ring and the gpsimd
            # SWDGE ring (2 queues -> ~2/3 of the round-robin DMA share)
            # while stores sit alone on the ACT ring: loads outpace
            # stores, so compute never starves and the store backlog
            # drains at full rate at the end instead of dribbling.
            if t % 2 == 0:
                nc.sync.dma_start(L[:], x[:][t])
            else:
                nc.gpsimd.dma_start(L[:], x[:][t])
            # S3 free layout: i*512 + s*16 + g*8 + j  (= i*512 + w)
            S3 = s3pool.tile([P, 4096], F32)
            for grp in range(GRPS):
                T1 = pt.tile([P, 512], F32)
                S1 = s1pool.tile([P, 512], BF16)
                O2 = po.tile([P, 512], F32)
                for d in range(4):
                    s = grp * 4 + d
                    nc.tensor.transpose(
                        T1[:, d * P : (d + 1) * P],
                        L[:, s * P : (s + 1) * P],
                        ident[:],
                    )
                # PSUM fp32 -> SBUF bf16: the cast rides the copy
                nc.vector.tensor_copy(S1[:], T1[:])
                for d in range(4):
                    nc.tensor.matmul(
                        O2[:, d * P : (d + 1) * P],
                        S1[:, d * P : (d + 1) * P],
                        g2t[:],
                        start=True,
                        stop=True,
                    )
                # copy O2 (free = dg*64+i*8+j per sub-tile d) into S3
                # at free = i*512 + (grp*4+d)*16 + g*8 + j; dg = d*2+g
                # merged (d,g) -> dg stride 8 in S3, stride 64 in O2.
                o2v = O2[:].rearrange("p (dg i j) -> p dg i j", dg=8, i=8, j=8)
                s3d = S3[:].rearrange(
                    "p (i grp dg j) -> p grp dg i j", i=8, grp=GRPS, dg=8, j=8
                )
                nc.scalar.copy(s3d[:, grp], o2v)
            # one fused store per tile: DRAM rows h = u*8+i, 512-float (2KB)
            # contiguous runs; SBUF partition p = im*64+u matches (im, u).
            nc.scalar.dma_start(outv[t], S3[:])

    nc.finalize()
    return nc


def _g2_matrix(idct_mat: np.ndarray) -> np.ndarray:
    m = np.asarray(idct_mat, dtype=np.float32)
    g = np.kron(m, m)  # g[(i,j),(k,m)] = M[i,k] * M[j,m]
    g2 = np.zeros((P, P), dtype=np.float32)
    g2[:64, :64] = g.T
    g2[64:, 64:] = g.T
    return g2


def _run(x_dct, idct_mat, H, W, trace: bool = False, tmpdir: str | None = None):
    import ml_dtypes

    x = np.ascontiguousarray(np.asarray(x_dct, dtype=np.float32))
    assert x.shape == (4, 64, 64, 64, BLOCK, BLOCK), x.shape
    H = int(H)
    W = int(W)
    assert H == 512 and W == 512, (H, W)

    g2 = _g2_matrix(idct_mat).astype(ml_dtypes.bfloat16)
    xs = x.reshape(N_CORES, TILES, P, 4096)

    nc = _build_nc(TILES)
    in_maps = [{"x": xs[c], "g2": g2} for c in range(N_CORES)]
    res = run_bass_kernel_spmd(
        nc, in_maps, core_ids=list(range(N_CORES)), trace=trace, tmpdir=tmpdir
    )
    outs = [res.results[c]["out"] for c in range(N_CORES)]
    full = np.concatenate(outs, axis=0).reshape(4, 64, 512, 512)
    return full[:, :, :H, :W], res


def kernel(x_dct, idct_mat=None, H=512, W=512):
    if idct_mat is None:
        idct_mat = _make_idct_matrix(BLOCK)
    out, _ = _run(x_dct, idct_mat, H, W, trace=False)
    return out


# revision 27
# speedup vs baseline: 1.0657x; 1.0657x over previous
"""Block 8x8 2D-IDCT kernel for Trainium2 (Bass/Tile), 8-core data-parallel.

Full input x_dct (4,64,64,64,8,8) f32 is sharded along flattened (N,C) into
8 shards of 32 images; each core independently computes the 2D IDCT of its
32 images and writes (32,512,512); results are concatenated on host.

Per-core pipeline, one tile = 2 images = 2 MiB = [128p x 4096] f32; each
partition p = (img, bh) holds one full block-row (32 block-pairs):
  DMA load (16KB/partition contiguous)
  -> PE transpose (float32r: 1.5 cyc/row, bit-exact pass-through) of 32x
     [128,128] sub-tiles (sub-tile s = block-pair column s; partitions
     become the 128 coeffs of the pair)
  -> DVE copy PSUM->SBUF casting to bf16
  -> bf16 matmul per sub-tile against G2 = blockdiag(G^T,G^T) in bf16,
     G = kron(M,M); fp32 PSUM accumulate:
       out[pair, 128 pixels of 2 blocks] in PSUM
  -> ACT copy PSUM->SBUF (fp32) permuted so free dim = (i, s, g, j) = (i, w)
  -> DMA store: 8 stores/tile, 256KB each, full 2KB DRAM rows
"""

import math
from contextlib import ExitStack

import numpy as np

import concourse.bass as bass
import concourse.mybir as mybir
import concourse.tile as tile
from concourse import bacc, masks
from concourse.bass_utils import run_bass_kernel_spmd

F32 = mybir.dt.float32
F32R = mybir.dt.float32r
BF16 = mybir.dt.bfloat16

N_CORES = 8
IMGS = 32           # images per core
TILES = IMGS // 2   # 2 images per tile
P = 128
SUBT = 32           # [128,128] sub-tiles per tile
GRPS = 8            # groups of 4 sub-tiles (one PSUM bank each)
BLOCK = 8


def _make_idct_matrix(nb: int) -> np.ndarray:
    m = np.zeros((nb, nb), dtype=np.float64)
    for n in range(nb):
        for k in range(nb):
            alpha = math.sqrt(1.0 / nb) if k == 0 else math.sqrt(2.0 / nb)
            m[n, k] = alpha * math.cos(math.pi * (2 * n + 1) * k / (2 * nb))
    return m.astype(np.float32)


def _build_nc(tiles: int = TILES) -> bass.Bass:
    nc = bacc.Bacc("TRN2", target_bir_lowering=False, debug=False)

    x = nc.dram_tensor("x", [tiles, P, 4096], F32, kind="ExternalInput")
    g2 = nc.dram_tensor("g2", [P, P], BF16, kind="ExternalInput")
    out = nc.dram_tensor("out", [2 * tiles, 512, 512], F32, kind="ExternalOutput")
    # out view: (t, im, u, i, w)
    outv = out[:].rearrange(
        "(t im) (u i) w -> t im u i w", t=tiles, im=2, u=64, i=8
    )

    with tile.TileContext(nc) as tc, ExitStack() as ctx:
        consts = ctx.enter_context(tc.tile_pool(name="consts", bufs=1))
        lpool = ctx.enter_context(tc.tile_pool(name="load", bufs=6))
        s1pool = ctx.enter_context(tc.tile_pool(name="s1", bufs=6))
        s3pool = ctx.enter_context(tc.tile_pool(name="s3", bufs=6))
        pt = ctx.enter_context(
            tc.tile_pool(name="pt", bufs=3, space=bass.MemorySpace.PSUM)
        )
        po = ctx.enter_context(
            tc.tile_pool(name="po", bufs=3, space=bass.MemorySpace.PSUM)
        )
        pw = ctx.enter_context(
            tc.tile_pool(name="pw", bufs=1, space=bass.MemorySpace.PSUM)
        )

        ident = consts.tile([P, P], F32)
        masks.make_identity(nc, ident[:])
        g2t = consts.tile([P, P], BF16)
        nc.sync.dma_start(g2t[:], g2[:])

        # PE warmup: ~5us of back-to-back matmuls so the systolic array
        # reaches full pstate before tile 0's transposes arrive (a cold
        # PE delays the first stores and cascades into DMA stalls).
        wu = pw.tile([P, P], F32)
        for _ in range(36):
            nc.tensor.matmul(wu[:], g2t[:], g2t[:], start=True, stop=True)

        pending = []
        for t in range(tiles):
            L = lpool.tile([P, 4096], F32)
            # loads on the SP HWDGE ring, stores on the ACT ring: separate
            # FIFO queues so a store waiting on compute never blocks a load
            nc.sync.dma_start(L[:], x[:][t])
            # S3 free layout: i*512 + s*16 + g*8 + j  (= i*512 + w)
            S3 = s3pool.tile([P, 4096], F32)
            for grp in range(GRPS):
                T1 = pt.tile([P, 512], F32)
                S1 = s1pool.tile([P, 512], BF16)
                O2 = po.tile([P, 512], F32)
                for d in range(4):
                    s = grp * 4 + d
                    nc.tensor.transpose(
                        T1[:, d * P : (d + 1) * P],
                        L[:, s * P : (s + 1) * P],
                        ident[:],
                    )
                # PSUM fp32 -> SBUF bf16: the cast rides the copy
                nc.vector.tensor_copy(S1[:], T1[:])
                for d in range(4):
                    nc.tensor.matmul(
                        O2[:, d * P : (d + 1) * P],
                        S1[:, d * P : (d + 1) * P],
                        g2t[:],
                        start=True,
                        stop=True,
                    )
                # copy O2 (free = dg*64+i*8+j per sub-tile d) into S3
                # at free = i*512 + (grp*4+d)*16 + g*8 + j; dg = d*2+g
                # merged (d,g) -> dg stride 8 in S3, stride 64 in O2.
                o2v = O2[:].rearrange("p (dg i j) -> p dg i j", dg=8, i=8, j=8)
                s3d = S3[:].rearrange(
                    "p (i grp dg j) -> p grp dg i j", i=8, grp=GRPS, dg=8, j=8
                )
                nc.scalar.copy(s3d[:, grp], o2v)
            # one fused store per tile: DRAM rows h = u*8+i, 512-float (2KB)
            # contiguous runs; SBUF partition p = im*64+u matches (im, u).
            # Stores are emitted 2 tiles late in the ACT stream: this keeps
            # a standing ~2-tile store backlog, so at the end of the run
            # the store ring drains continuously at full rate instead of
            # idling between the last tiles' compute completions.
            pending.append((t, S3))
            if len(pending) > 2:
                tp, S3p = pending.pop(0)
                nc.scalar.dma_start(outv[tp], S3p[:])
        for tp, S3p in pending:
            nc.scalar.dma_start(outv[tp], S3p[:])

    nc.finalize()
    return nc


def _g2_matrix(idct_mat: np.ndarray) -> np.ndarray:
    m = np.asarray(idct_mat, dtype=np.float32)
    g = np.kron(m, m)  # g[(i,j),(k,m)] = M[i,k] * M[j,m]
    g2 = np.zeros((P, P), dtype=np.float32)
    g2[:64, :64] = g.T
    g2[64:, 64:] = g.T
    return g2


def _run(x_dct, idct_mat, H, W, trace: bool = False, tmpdir: str | None = None):
    import ml_dtypes

    x = np.ascontiguousarray(np.asarray(x_dct, dtype=np.float32))
    assert x.shape == (4, 64, 64, 64, BLOCK, BLOCK), x.shape
    H = int(H)
    W = int(W)
    assert H == 512 and W == 512, (H, W)

    g2 = _g2_matrix(idct_mat).astype(ml_dtypes.bfloat16)
    xs = x.reshape(N_CORES, TILES, P, 4096)

    nc = _build_nc(TILES)
    in_maps = [{"x": xs[c], "g2": g2} for c in range(N_CORES)]
    res = run_bass_kernel_spmd(
        nc, in_maps, core_ids=list(range(N_CORES)), trace=trace, tmpdir=tmpdir
    )
    outs = [res.results[c]["out"] for c in range(N_CORES)]
    full = np.concatenate(outs, axis=0).reshape(4, 64, 512, 512)
    return full[:, :, :H, :W], res


def kernel(x_dct, idct_mat=None, H=512, W=512):
    if idct_mat is None:
        idct_mat = _make_idct_matrix(BLOCK)
    out, _ = _run(x_dct, idct_mat, H, W, trace=False)
    return out
